# revision 23
# baseline (speedup 1.0000x reference)
"""MHA forward (dense transformer block) for TRN2, 8 NeuronCores.

Sharding: core c handles batch b = c // 4 and head-group g = c % 4
(4 heads of 64 dims = 256 hidden columns).  Wq/Wkv are sharded
column-wise, Wo row-wise; each core produces a partial [2048, 1024]
output which the host sums per batch (+ bo).

v6 design (all-fp16, measured-cost-driven):
  - q staged by DMA and transposed on the PE (fp16, 1.0 cyc/row); the
    DMA-XBAR transpose path measured 42us/rep serialized - too slow.
  - All matmul chains use N=512 moving operands so the per-matmul
    LDWEIGHTS (~P/1.2 ns) hides under the 213 ns streams (proj chains
    measured 238 ns/MM, scores 200 ns/MM = the PSUM write-port floor).
  - Scores: 2 heads packed per 128 partitions; both heads share one
    PSUM tile per key-block ([128, 2h, 512] = 2 banks, pst bufs=3)
    giving a 3-deep score->exp pipeline with one activation call per
    key-block.
  - attn@V: column-tiled - h0's V chain writes PSUM partitions 0-63
    (tile (0,0)), h1's 64-127 (tile (0,64)); the two streams run
    concurrently (measured 94 ns/MM effective vs 243 serial).  A second
    col-tiled chain pair against an all-ones lhsT accumulates the
    softmax denominators pre-broadcast over each head's 64 partitions;
    reciprocal_approx_fast + one tensor_mul normalize both heads.
  - Emission order interleaves projection/output chains into the
    attention pipeline so the ACT exp stream never starves.
"""

import sys

sys.path.insert(0, "/opt/trn_rl_repo")

import numpy as np

import concourse.bass as bass
from concourse import bacc
import concourse.mybir as mybir
import concourse.tile as tile

F32 = mybir.dt.float32
F16 = mybir.dt.float16
AF = mybir.ActivationFunctionType

S = 2048          # sequence length per batch
D = 1024          # model dim
DH = 64           # head dim
NH = 4            # heads per core
GH = 256          # hidden cols per core
VA = DH + 1       # V' cols per head (ones column appended)
KC = D // 128     # 8 contraction chunks of 128
ST = S // 128     # 16 sequence blocks of 128
SPAN = 512        # q-span per scores/exp/attnV block
NSP = S // SPAN   # 4 spans
SCALE = DH ** -0.5

N_CORES = 8


def _build_nc(reps=1):
    nc = bacc.Bacc("TRN2", target_bir_lowering=False)

    qb = nc.declare_dram_parameter("qb", [S, D], F16, isOutput=False)
    wq = nc.declare_dram_parameter("wq", [D, GH], F16, isOutput=False)
    wk = nc.declare_dram_parameter("wk", [D, GH], F16, isOutput=False)
    wv = nc.declare_dram_parameter("wv", [D, GH], F16, isOutput=False)
    bqk = nc.declare_dram_parameter("bqk", [128, 4], F32, isOutput=False)
    bvb = nc.declare_dram_parameter("bvb", [128, GH], F32, isOutput=False)
    wo = nc.declare_dram_parameter("wo", [GH, D], F16, isOutput=False)
    out = nc.declare_dram_parameter("partial", [S, D], F16, isOutput=True)

    with tile.TileContext(nc) as tc:
        with (
            tc.tile_pool(name="wsing", bufs=1) as wsing,
            tc.tile_pool(name="qstage", bufs=2) as qstage,
            tc.tile_pool(name="big", bufs=8) as big,
            tc.tile_pool(name="qk", bufs=6) as qk,
            tc.tile_pool(name="ptp", bufs=3) as ptp,
            tc.tile_pool(name="vp", bufs=16) as vp,
            tc.tile_pool(name="small", bufs=2) as small,
            tc.tile_pool(name="ostage", bufs=2) as ostage,
            tc.tile_pool(name="pmm", bufs=2, space="PSUM") as pmm,
            tc.tile_pool(name="pst", bufs=3, space="PSUM") as pst,
        ):
            ident_f = wsing.tile([128, 128], F32, name="ident_f")
            from concourse.masks import make_identity
            make_identity(nc, ident_f)
            ident = wsing.tile([128, 128], F16)
            nc.vector.tensor_copy(ident, ident_f)
            ones_f = wsing.tile([128, 64], F32, name="ones_f")
            nc.vector.memset(ones_f, 1.0)
            ones64 = wsing.tile([128, 64], F16)
            nc.vector.tensor_copy(ones64, ones_f)

            wq_t = wsing.tile([128, KC, GH], F16)
            nc.sync.dma_start(out=wq_t, in_=wq[:, :].rearrange("(kc p) f -> p kc f", p=128))
            wk_t = wsing.tile([128, KC, GH], F16)
            nc.sync.dma_start(out=wk_t, in_=wk[:, :].rearrange("(kc p) f -> p kc f", p=128))
            wv_t = wsing.tile([128, KC, GH], F16)
            nc.sync.dma_start(out=wv_t, in_=wv[:, :].rearrange("(kc p) f -> p kc f", p=128))
            wo_t = wsing.tile([128, 2, D], F16)
            nc.sync.dma_start(out=wo_t, in_=wo[:, :].rearrange("(c p) f -> p c f", p=128))
            bqk_t = wsing.tile([128, 4], F32)
            nc.sync.dma_start(out=bqk_t, in_=bqk[:, :])
            bvb_t = wsing.tile([128, GH], F32)
            nc.sync.dma_start(out=bvb_t, in_=bvb[:, :])

            consts = (qb, out, wq_t, wk_t, wv_t, wo_t, bqk_t, bvb_t, ident, ones64)
            pools = (qstage, big, qk, ptp, vp, small, ostage, pmm, pst)

            def emit_body():
                _emit_body(nc, consts, pools)

            if reps == 1:
                emit_body()
            else:
                with tc.For_i(0, reps, 1):
                    emit_body()

    nc.compile()
    return nc


def _emit_body(nc, consts, pools):
    (qb, out, wq_t, wk_t, wv_t, wo_t, bqk_t, bvb_t, ident, ones64) = consts
    (qstage, big, qk, ptp, vp, small, ostage, pmm, pst) = pools

    qT = [big.tile([128, S], F16, tag="big", name=f"qT{j}") for j in range(KC)]
    KT = [qk.tile([128, S], F16, tag="qk", name=f"KT{p}") for p in range(2)]
    QT = [qk.tile([128, S], F16, tag="qk", name=f"QT{p}") for p in range(2)]
    OT = [qk.tile([128, S], F16, tag="qk", name=f"OT{p}") for p in range(2)]
    vpr = [None] * ST
    pts = {}

    # ---- emission units ----
    def TR(tg):
        """DMA + PE-transpose task-group tg (512 q rows) into qT columns."""
        qs = []
        for half in range(2):
            t2 = tg * 2 + half
            q_tile = qstage.tile([128, 2, D], F16, tag="qs", name=f"qs{tg}_{half}")
            nc.sync.dma_start(
                out=q_tile,
                in_=qb[t2 * 256:(t2 + 1) * 256, :].rearrange(
                    "(tt p) d -> p tt d", p=128),
            )
            qs.append(q_tile)
        for j in range(KC):
            ps = pmm.tile([128, 512], F16, tag="mm", name=f"trp{tg}_{j}")
            for tt in range(4):
                nc.tensor.transpose(
                    ps[:, tt * 128:(tt + 1) * 128],
                    qs[tt // 2][:, tt % 2, j * 128:(j + 1) * 128],
                    ident,
                )
            nc.vector.tensor_copy(qT[j][:, tg * 512:(tg + 1) * 512], ps)

    def KCH(pair, sp4):
        _qk_chain(KT, wk_t, 2, pair, sp4)

    def QCH(pair, sp4):
        _qk_chain(QT, wq_t, 0, pair, sp4)

    def _qk_chain(dst_list, w_t, bias_base, pair, sp4):
        ps = pmm.tile([128, 512], F32, tag="mm", name=f"qkc{bias_base}_{pair}_{sp4}")
        for k in range(KC):
            nc.tensor.matmul(
                ps,
                w_t[:, k, pair * 128:(pair + 1) * 128],
                qT[k][:, sp4 * 512:(sp4 + 1) * 512],
                start=(k == 0),
                stop=(k == KC - 1),
            )
        nc.vector.tensor_scalar_add(
            dst_list[pair][:, sp4 * 512:(sp4 + 1) * 512],
            ps,
            bqk_t[:, bias_base + pair:bias_base + pair + 1],
        )

    def VCH(sb):
        """V projection for key-block sb (natural 4x64 head layout)."""
        ps = pmm.tile([128, GH], F32, tag="mm", name=f"vc{sb}")
        for k in range(KC):
            nc.tensor.matmul(
                ps,
                qT[k][:, sb * 128:(sb + 1) * 128],
                wv_t[:, k, :],
                start=(k == 0),
                stop=(k == KC - 1),
            )
        v_tile = vp.tile([128, GH], F16, tag="vp", name=f"vpr{sb}")
        nc.vector.tensor_add(v_tile, ps, bvb_t)
        vpr[sb] = v_tile

    def SX(pair, sp, kb):
        """Scores + exp for one key-block kb of span sp: both heads share
        one PSUM tile ([128, 2h, 512] = 2 banks) and one activation call,
        so pst bufs=3 gives a 3-deep score/exp pipeline."""
        q0 = sp * SPAN
        if kb == 0:
            pts[(pair, sp)] = ptp.tile(
                [128, 2, ST, SPAN], F16, tag="pt", name=f"pt{pair}_{sp}")
        pt = pts[(pair, sp)]
        sts = pst.tile([128, 2, SPAN], F32, tag="st", name=f"st{pair}_{sp}_{kb}")
        for h in range(2):
            nc.tensor.matmul(
                sts[:, h, :],
                KT[pair][h * 64:(h + 1) * 64, kb * 128:(kb + 1) * 128],
                QT[pair][h * 64:(h + 1) * 64, q0:q0 + SPAN],
                start=True,
                stop=True,
            )
        nc.scalar.activation(
            pt[:, :, kb, :],
            sts,
            AF.Exp,
            scale=SCALE,
        )

    avt = {}

    def AVh(pair, sp, half):
        """attn@V + denominator for span sp, key-blocks half*8..half*8+7.

        Column-tiled: h0's V chain writes PSUM partitions 0-63 (tile (0,0)),
        h1's writes 64-127 (tile (0,64)), so the two streams run
        concurrently (measured 94 ns/MM effective vs 243 serial).  A second
        pair of chains against a static all-ones lhsT accumulates the
        softmax denominators, pre-broadcast across the 64 partitions each
        head's dims occupy - no gpsimd broadcast needed."""
        pt = pts[(pair, sp)]
        if half == 0:
            avt[(pair, sp)] = (
                pmm.tile([128, SPAN], F32, tag="mm", name=f"ov{pair}_{sp}"),
                pmm.tile([128, SPAN], F32, tag="mm", name=f"dn{pair}_{sp}"),
            )
        ov, dns = avt[(pair, sp)]
        for kb in range(half * 8, half * 8 + 8):
            for h in range(2):
                hh = pair * 2 + h
                nc.tensor.matmul(
                    ov[h * 64:(h + 1) * 64, :],
                    vpr[kb][:, hh * 64:(hh + 1) * 64],
                    pt[:, h, kb, :],
                    start=(kb == 0),
                    stop=(kb == ST - 1),
                    skip_group_check=True,
                )
            for h in range(2):
                nc.tensor.matmul(
                    dns[h * 64:(h + 1) * 64, :],
                    ones64,
                    pt[:, h, kb, :],
                    start=(kb == 0),
                    stop=(kb == ST - 1),
                    skip_group_check=True,
                )
        if half == 1:
            rdb = small.tile([128, SPAN], F32, tag="rdb", name=f"rdb{pair}_{sp}")
            nc.vector.reciprocal_approx_fast(rdb, dns)
            nc.vector.tensor_mul(
                OT[pair][:, sp * SPAN:(sp + 1) * SPAN],
                ov,
                rdb,
            )
            pts.pop((pair, sp))
            avt.pop((pair, sp))

    def OP(sb):
        """Output projection + store for seq block sb (128 rows)."""
        o_tile = ostage.tile([128, D], F16, tag="ostage", name=f"ot{sb}")
        for n in range(2):
            ps = pmm.tile([128, 512], F32, tag="mm", name=f"op{sb}_{n}")
            for c in range(2):
                nc.tensor.matmul(
                    ps,
                    OT[c][:, sb * 128:(sb + 1) * 128],
                    wo_t[:, c, n * 512:(n + 1) * 512],
                    start=(c == 0),
                    stop=(c == 1),
                )
            nc.vector.tensor_copy(o_tile[:, n * 512:(n + 1) * 512], ps)
        nc.sync.dma_start(out=out[sb * 128:(sb + 1) * 128, :], in_=o_tile)

    # ---- master emission order (software pipeline) ----
    TR(0); TR(1)
    KCH(0, 0); QCH(0, 0)
    SX(0, 0, 0); SX(0, 0, 1); SX(0, 0, 2); SX(0, 0, 3)
    KCH(0, 1)
    TR(2); SX(0, 0, 4); SX(0, 0, 5)
    KCH(0, 2); SX(0, 0, 6); SX(0, 0, 7)
    TR(3); SX(0, 0, 8); SX(0, 0, 9)
    KCH(0, 3); SX(0, 0, 10); SX(0, 0, 11)
    QCH(0, 1); SX(0, 0, 12); SX(0, 0, 13)
    VCH(0); VCH(1); SX(0, 0, 14); SX(0, 0, 15)
    VCH(2); VCH(3); SX(0, 1, 0); SX(0, 1, 1)
    VCH(4); VCH(5); SX(0, 1, 2); SX(0, 1, 3)
    VCH(6); VCH(7); SX(0, 1, 4); SX(0, 1, 5)
    VCH(8); VCH(9); SX(0, 1, 6); SX(0, 1, 7)
    VCH(10); VCH(11); SX(0, 1, 8); SX(0, 1, 9)
    VCH(12); VCH(13); SX(0, 1, 10); SX(0, 1, 11)
    VCH(14); VCH(15); SX(0, 1, 12); SX(0, 1, 13)
    QCH(0, 2); SX(0, 1, 14); SX(0, 1, 15)
    AVh(0, 0, 0); SX(0, 2, 0); SX(0, 2, 1)
    AVh(0, 0, 1); SX(0, 2, 2); SX(0, 2, 3)
    QCH(0, 3); SX(0, 2, 4); SX(0, 2, 5)
    KCH(1, 0); SX(0, 2, 6); SX(0, 2, 7)
    KCH(1, 1); SX(0, 2, 8); SX(0, 2, 9)
    AVh(0, 1, 0); SX(0, 2, 10); SX(0, 2, 11)
    AVh(0, 1, 1); SX(0, 2, 12); SX(0, 2, 13)
    KCH(1, 2); SX(0, 2, 14); SX(0, 2, 15)
    KCH(1, 3); SX(0, 3, 0); SX(0, 3, 1)
    QCH(1, 0); SX(0, 3, 2); SX(0, 3, 3)
    AVh(0, 2, 0); SX(0, 3, 4); SX(0, 3, 5)
    AVh(0, 2, 1); SX(0, 3, 6); SX(0, 3, 7)
    QCH(1, 1); SX(0, 3, 8); SX(0, 3, 9)
    QCH(1, 2); SX(0, 3, 10); SX(0, 3, 11)
    QCH(1, 3); SX(0, 3, 12); SX(0, 3, 13)
    SX(0, 3, 14); SX(0, 3, 15)
    AVh(0, 3, 0); SX(1, 0, 0); SX(1, 0, 1)
    AVh(0, 3, 1); SX(1, 0, 2); SX(1, 0, 3)
    SX(1, 0, 4); SX(1, 0, 5); SX(1, 0, 6); SX(1, 0, 7)
    SX(1, 0, 8); SX(1, 0, 9); SX(1, 0, 10); SX(1, 0, 11)
    SX(1, 0, 12); SX(1, 0, 13); SX(1, 0, 14); SX(1, 0, 15)
    AVh(1, 0, 0); SX(1, 1, 0); SX(1, 1, 1)
    AVh(1, 0, 1); SX(1, 1, 2); SX(1, 1, 3)
    OP(0); SX(1, 1, 4); SX(1, 1, 5)
    OP(1); SX(1, 1, 6); SX(1, 1, 7)
    OP(2); SX(1, 1, 8); SX(1, 1, 9)
    OP(3); SX(1, 1, 10); SX(1, 1, 11)
    SX(1, 1, 12); SX(1, 1, 13); SX(1, 1, 14); SX(1, 1, 15)
    AVh(1, 1, 0); SX(1, 2, 0); SX(1, 2, 1)
    AVh(1, 1, 1); SX(1, 2, 2); SX(1, 2, 3)
    OP(4); SX(1, 2, 4); SX(1, 2, 5)
    OP(5); SX(1, 2, 6); SX(1, 2, 7)
    OP(6); SX(1, 2, 8); SX(1, 2, 9)
    OP(7); SX(1, 2, 10); SX(1, 2, 11)
    SX(1, 2, 12); SX(1, 2, 13); SX(1, 2, 14); SX(1, 2, 15)
    AVh(1, 2, 0); SX(1, 3, 0); SX(1, 3, 1)
    AVh(1, 2, 1); SX(1, 3, 2); SX(1, 3, 3)
    OP(8); SX(1, 3, 4); SX(1, 3, 5)
    OP(9); SX(1, 3, 6); SX(1, 3, 7)
    OP(10); SX(1, 3, 8); SX(1, 3, 9)
    OP(11); SX(1, 3, 10); SX(1, 3, 11)
    SX(1, 3, 12); SX(1, 3, 13); SX(1, 3, 14); SX(1, 3, 15)
    AVh(1, 3, 0); AVh(1, 3, 1)
    OP(12); OP(13); OP(14); OP(15)


_NC_CACHE = {}


def _get_nc(reps=1):
    if reps not in _NC_CACHE:
        _NC_CACHE[reps] = _build_nc(reps)
    return _NC_CACHE[reps]


def _shard_inputs(q, Wq, bq, Wkv, bkv, Wo, bo):
    q = np.asarray(q, dtype=np.float32)
    Wq = np.asarray(Wq, dtype=np.float32)
    bq = np.asarray(bq, dtype=np.float32)
    Wkv = np.asarray(Wkv, dtype=np.float32)
    bkv = np.asarray(bkv, dtype=np.float32)
    Wo = np.asarray(Wo, dtype=np.float32)

    HID = D  # 1024 total hidden
    in_maps = []
    for c in range(N_CORES):
        b, g = divmod(c, 4)
        lo = g * GH
        wk_s = Wkv[:, lo:lo + GH]
        wv_s = Wkv[:, HID + lo:HID + lo + GH]
        bq_s = bq[lo:lo + GH]
        bk_s = bkv[lo:lo + GH]
        bv_s = bkv[HID + lo:HID + lo + GH]

        bqk_pp = np.stack(
            [bq_s[0:128], bq_s[128:256], bk_s[0:128], bk_s[128:256]], axis=1
        )

        in_maps.append({
            "qb": np.ascontiguousarray(q[b]).astype(np.float16),
            "wq": np.ascontiguousarray(Wq[:, lo:lo + GH]).astype(np.float16),
            "wk": np.ascontiguousarray(wk_s).astype(np.float16),
            "wv": np.ascontiguousarray(wv_s).astype(np.float16),
            "bqk": np.ascontiguousarray(bqk_pp),
            "bvb": np.broadcast_to(bv_s, (128, GH)).copy(),
            "wo": np.ascontiguousarray(Wo[lo:lo + GH, :]).astype(np.float16),
        })
    return in_maps


def _gather(results, bo):
    bo = np.asarray(bo, dtype=np.float32)
    out = np.empty((2, S, D), dtype=np.float32)
    for b in range(2):
        acc = results[4 * b]["partial"].astype(np.float32)
        for g in range(1, 4):
            acc = acc + results[4 * b + g]["partial"].astype(np.float32)
        out[b] = acc + bo
    return out


_RUNNER_CACHE = {}


def _make_runner(reps=1):
    """Build (once) a reusable jitted SPMD callable for the given rep count.

    Re-jitting per call loads a second copy of the NEFF and has been seen to
    wedge the exec unit, so the jitted executable is cached per process.
    """
    if reps in _RUNNER_CACHE:
        return _RUNNER_CACHE[reps]

    import jax
    from jax.sharding import Mesh, PartitionSpec
    from jax.experimental.shard_map import shard_map
    from concourse import bass2jax

    nc = _get_nc(reps)
    bass2jax.install_neuronx_cc_hook()
    partition_name = nc.partition_id_tensor.name if nc.partition_id_tensor else None
    in_names, out_names, out_avals, zero_outs = [], [], [], []
    for alloc in nc.m.functions[0].allocations:
        if not isinstance(alloc, mybir.MemoryLocationSet):
            continue
        name = alloc.memorylocations[0].name
        if alloc.kind == "ExternalInput":
            if name != partition_name:
                in_names.append(name)
        elif alloc.kind == "ExternalOutput":
            out_names.append(name)
            shape = tuple(alloc.tensor_shape)
            dtype = mybir.dt.np(alloc.dtype)
            out_avals.append(jax.core.ShapedArray(shape, dtype))
            zero_outs.append(np.zeros(shape, dtype))
    n_params = len(in_names)
    n_outs = len(out_avals)
    in_names.extend(out_names)
    if partition_name:
        in_names.append(partition_name)

    def _body(*args):
        operands = list(args)
        if partition_name:
            operands.append(bass2jax.partition_id_tensor())
        return tuple(bass2jax._bass_exec_p.bind(
            *operands,
            out_avals=tuple(out_avals),
            in_names=tuple(in_names),
            out_names=tuple(out_names),
            lowering_input_output_aliases=(),
            sim_require_finite=True,
            sim_require_nnan=True,
            nc=nc,
        ))

    devices = jax.devices()[:N_CORES]
    mesh = Mesh(np.asarray(devices), ("core",))
    donate = tuple(range(n_params, n_params + n_outs))
    sharded = jax.jit(
        shard_map(_body, mesh=mesh,
                  in_specs=(PartitionSpec("core"),) * (n_params + n_outs),
                  out_specs=(PartitionSpec("core"),) * len(out_names),
                  check_rep=False),
        donate_argnums=donate, keep_unused=True)

    def run(in_maps):
        per_core = [[np.asarray(m[nm]) for nm in in_names[:n_params]]
                    for m in in_maps]
        concat_in = [np.concatenate([per_core[c][i] for c in range(N_CORES)],
                                    axis=0) for i in range(n_params)]
        zo = [np.concatenate([z] * N_CORES, axis=0) for z in zero_outs]
        outs = sharded(*concat_in, *zo)
        outs = [np.asarray(o) for o in outs]
        per_core_res = []
        for c in range(N_CORES):
            per_core_res.append({
                name: np.split(outs[i], N_CORES, axis=0)[c]
                for i, name in enumerate(out_names)
            })
        return per_core_res

    _RUNNER_CACHE[reps] = run
    return run


def _run(inputs, reps=1):
    run = _make_runner(reps)
    in_maps = _shard_inputs(**inputs)
    results = run(in_maps)
    out = _gather(results, inputs["bo"])
    return out, results


def kernel(q, Wq, bq, Wkv, bkv, Wo, bo):
    out, _ = _run(dict(q=q, Wq=Wq, bq=bq, Wkv=Wkv, bkv=bkv, Wo=Wo, bo=bo))
    return out
